# revision 6
# baseline (speedup 1.0000x reference)
"""Trainium2 Bass kernel for an equivariant-GNN style network.

Reference computation (per node, then per-graph mean pool):
    x  = concat([pos, emb[z]])           # [N, 128]
    x1 = relu(x @ W1 + b1)               # [N, 128]
    x2 = relu(x1 @ W2 + b2)              # [N, 128]
    y  = x2 @ W3 + b3                    # [N, 64]
    out[g] = mean over nodes of graph g of y   # [4096, 64]

Strategy (8 NeuronCores, SPMD, one program):
  - Nodes sharded across cores at graph boundaries (batch is sorted): no
    cross-core collective; each core emits the means for its graph range.
  - Feature-major layout ([feat_partitions, nodes_free]), 512-node tiles:
      * emb[z] @ W1[3:] folded into one matmul with an on-chip one-hot(z)
        rhs (PE broadcast of z in bf16 + DVE is_equal); pos rows appended
        to the same rhs -> layer 1 is ONE fp32r matmul.
      * layers 1-2 in fp32r (1 col/cycle, ~2^-11 product rounding).
      * layer 3 via x2 (bf16) as the stationary operand against a stacked
        [W3_hi | W3_lo] bf16 rhs -> node-major y with no systematic W3
        rounding error; hi+lo summed during the PSUM->SBUF cast.
      * segment mean pool = matmul with an exact bf16 one-hot segment
        matrix (batch ids vs a constant index ramp, computed on GPSIMD),
        PSUM-accumulated, scaled by host-precomputed 1/count.
  - b3 and 1/count folded on the host (O(4096) work).

The walrus in this toolchain only allows ONE sync wait per instruction;
_split_multi_waits() rewrites Tile's multi-wait instructions into NoOp
chains before compilation.
"""

import os
import sys
from contextlib import ExitStack

import numpy as np

if "/opt/trn_rl_repo" not in sys.path:
    sys.path.insert(0, "/opt/trn_rl_repo")

import ml_dtypes
import concourse.bass as bass
import concourse.tile as tile
from concourse import mybir
from concourse.bass_utils import run_bass_kernel_spmd

N_CORES = 8
NT = 512        # nodes per tile (fp32 PSUM bank = 512 floats free dim)
SUB = 128       # nodes per sub-tile (matmul M limit)
GT = 128        # graphs per graph tile
N_TYPES = 100
HID = 128
OUT_DIM = 64
F32 = mybir.dt.float32
F32R = mybir.dt.float32r
BF16 = mybir.dt.bfloat16
NPBF = ml_dtypes.bfloat16

LAST_RESULTS = None  # BassKernelResults of the most recent run (for test.py)


def _split_multi_waits(nc):
    """The TPB ISA has one sync-wait slot per instruction and this walrus
    refuses to lower instructions with more. Move all-but-one wait onto
    same-engine NoOps placed immediately before (engines execute their queue
    in order, so the sync semantics are preserved)."""
    n_split = 0
    for f in nc.m.functions:
        for blk in f.blocks:
            insts = list(blk.instructions)
            out = []
            for inst in insts:
                si = inst.sync_info
                if si is not None and si.on_wait and len(si.on_wait) > 1:
                    waits = list(si.on_wait)
                    for j, w in enumerate(waits[:-1]):
                        nop = mybir.InstNoOp(
                            name=f"{inst.name}-w{j}",
                            engine=inst.engine,
                            ins=[],
                            outs=[],
                            sync_info=mybir.SyncInfo(on_wait=[w], on_update=[]),
                        )
                        out.append(nop)
                    inst.sync_info = mybir.SyncInfo(
                        on_wait=[waits[-1]], on_update=list(si.on_update or [])
                    )
                    n_split += 1
                out.append(inst)
            blk.instructions = out
    return n_split


def _build_bass(n_tiles: int, ntg: int, sched) -> bass.Bass:
    """Build the single SPMD program.

    sched[i] = list of (graph_tile, [sub_tiles...]) pooling work for node
    tile i; identical for all cores (union over cores, extra work is a no-op
    because the one-hot segment matrix vanishes for non-matching rows).
    """
    nc = bass.Bass()
    nc_pad = n_tiles * NT

    posr = nc.dram_tensor("posr", [3, nc_pad], F32R, kind="ExternalInput")
    zbf = nc.dram_tensor("zbf", [1, nc_pad], BF16, kind="ExternalInput")
    batch2d = nc.dram_tensor("batch2d", [SUB, nc_pad // SUB], F32, kind="ExternalInput")
    w1e = nc.dram_tensor("w1e", [N_TYPES + 3, HID], F32R, kind="ExternalInput")
    w2 = nc.dram_tensor("w2", [HID, HID], F32R, kind="ExternalInput")
    w3hl = nc.dram_tensor("w3hl", [HID, 2 * OUT_DIM], BF16, kind="ExternalInput")
    b1 = nc.dram_tensor("b1", [HID, 1], F32, kind="ExternalInput")
    b2 = nc.dram_tensor("b2", [HID, 1], F32, kind="ExternalInput")
    iota_col = nc.dram_tensor("iota_col", [SUB, 1], F32, kind="ExternalInput")
    f2 = nc.dram_tensor("f2", [SUB, ntg * GT], F32, kind="ExternalInput")
    invc = nc.dram_tensor("invc", [GT, ntg], F32, kind="ExternalInput")
    out = nc.dram_tensor("out", [ntg * GT, OUT_DIM], F32, kind="ExternalOutput")

    n_sub = NT // SUB

    with tile.TileContext(nc) as tc, ExitStack() as ctx:
        singles = ctx.enter_context(tc.tile_pool(name="singles", bufs=1))
        inp = ctx.enter_context(tc.tile_pool(name="inp", bufs=3))
        ohp = ctx.enter_context(tc.tile_pool(name="ohp", bufs=2))
        acts = ctx.enter_context(tc.tile_pool(name="acts", bufs=2))
        sgp = ctx.enter_context(tc.tile_pool(name="sgp", bufs=4))
        resp = ctx.enter_context(tc.tile_pool(name="resp", bufs=2))
        ps_zb = ctx.enter_context(tc.tile_pool(name="ps_zb", bufs=2, space="PSUM"))
        ps_x1 = ctx.enter_context(tc.tile_pool(name="ps_x1", bufs=2, space="PSUM"))
        ps_x2 = ctx.enter_context(tc.tile_pool(name="ps_x2", bufs=1, space="PSUM"))
        ps_y = ctx.enter_context(tc.tile_pool(name="ps_y", bufs=1, space="PSUM"))
        ps_pl = ctx.enter_context(tc.tile_pool(name="ps_pl", bufs=2, space="PSUM"))

        # ---- persistent tiles -------------------------------------------
        w1e_sb = singles.tile([N_TYPES + 3, HID], F32R)
        nc.sync.dma_start(out=w1e_sb, in_=w1e[:, :])
        w2_sb = singles.tile([HID, HID], F32R)
        nc.sync.dma_start(out=w2_sb, in_=w2[:, :])
        w3_sb = singles.tile([HID, 2 * OUT_DIM], BF16)
        nc.sync.dma_start(out=w3_sb, in_=w3hl[:, :])
        b1_sb = singles.tile([HID, 1], F32)
        nc.sync.dma_start(out=b1_sb, in_=b1[:, :])
        b2_sb = singles.tile([HID, 1], F32)
        nc.sync.dma_start(out=b2_sb, in_=b2[:, :])
        iota_sb = singles.tile([SUB, 1], F32)
        nc.sync.dma_start(out=iota_sb, in_=iota_col[:, :])
        f2_sb = singles.tile([SUB, ntg * GT], F32)
        nc.sync.dma_start(out=f2_sb, in_=f2[:, :])
        invc_sb = singles.tile([GT, ntg], F32)
        nc.sync.dma_start(out=invc_sb, in_=invc[:, :])
        ones_sb = singles.tile([1, N_TYPES], BF16)
        nc.vector.memset(ones_sb, 1.0)
        poolacc = singles.tile([GT, ntg * OUT_DIM], F32)
        nc.vector.memset(poolacc, 0.0)

        # ---- node tile loop ---------------------------------------------
        for i in range(n_tiles):
            # rhs for layer 1: rows 0..99 = one-hot(z), rows 100..102 = pos
            oh = ohp.tile([N_TYPES + 3, NT], F32R)
            nc.sync.dma_start(
                out=oh[N_TYPES : N_TYPES + 3, :], in_=posr[:, i * NT : (i + 1) * NT]
            )
            zf_t = inp.tile([1, NT], BF16, tag="zf")
            nc.sync.dma_start(out=zf_t, in_=zbf[:, i * NT : (i + 1) * NT])
            bt = inp.tile([SUB, n_sub], F32, tag="bt")
            nc.sync.dma_start(out=bt, in_=batch2d[:, i * n_sub : (i + 1) * n_sub])

            # broadcast z across 100 partitions (bf16 outer product, exact)
            zb = ps_zb.tile([N_TYPES, NT], F32)
            nc.tensor.matmul(zb, ones_sb, zf_t, start=True, stop=True)
            # one-hot: oh[t, n] = (z[n] == t)
            nc.vector.tensor_scalar(
                oh[:N_TYPES, :], zb, iota_sb[:N_TYPES, :], None, mybir.AluOpType.is_equal
            )

            # layer 1 (embedding fold + pos): x1 = relu(W1e.T @ oh + b1)
            x1p = ps_x1.tile([HID, NT], F32)
            nc.tensor.matmul(x1p, w1e_sb, oh, start=True, stop=True)
            x1 = acts.tile([HID, NT], F32R, tag="x1")
            nc.scalar.activation(x1, x1p, mybir.ActivationFunctionType.Relu, bias=b1_sb)

            # layer 2
            x2p = ps_x2.tile([HID, NT], F32)
            nc.tensor.matmul(x2p, w2_sb, x1, start=True, stop=True)
            x2 = acts.tile([HID, NT], BF16, tag="x2")
            nc.scalar.activation(x2, x2p, mybir.ActivationFunctionType.Relu, bias=b2_sb)

            # layer 3, node-major: yp[:, s] = x2_s.T @ W3h + x2_s.T @ W3l
            # (hi+lo accumulated in PSUM -> no systematic W3 rounding error)
            yp = ps_y.tile([SUB, n_sub, OUT_DIM], F32)
            for s in range(n_sub):
                nc.tensor.matmul(
                    yp[:, s, :],
                    x2[:, s * SUB : (s + 1) * SUB],
                    w3_sb[:, :OUT_DIM],
                    start=True,
                    stop=False,
                )
                nc.tensor.matmul(
                    yp[:, s, :],
                    x2[:, s * SUB : (s + 1) * SUB],
                    w3_sb[:, OUT_DIM:],
                    start=False,
                    stop=True,
                )
            y = acts.tile([SUB, n_sub, OUT_DIM], BF16, tag="y")
            nc.vector.tensor_copy(y, yp)

            # segment pooling: psum[g, f] += sum_n (batch[n] == g) * y[n, f]
            for t, ss in sched[i]:
                pp = ps_pl.tile([GT, OUT_DIM], F32)
                for k, s in enumerate(ss):
                    sg = sgp.tile([SUB, GT], BF16)
                    nc.gpsimd.tensor_scalar(
                        sg,
                        f2_sb[:, t * GT : (t + 1) * GT],
                        bt[:, s : s + 1],
                        None,
                        mybir.AluOpType.is_equal,
                    )
                    nc.tensor.matmul(
                        pp,
                        sg,
                        y[:, s, :],
                        start=(k == 0),
                        stop=(k == len(ss) - 1),
                    )
                nc.vector.tensor_add(
                    poolacc[:, t * OUT_DIM : (t + 1) * OUT_DIM],
                    poolacc[:, t * OUT_DIM : (t + 1) * OUT_DIM],
                    pp,
                )

        # ---- finalize: scale by 1/count, write out ----------------------
        for t in range(ntg):
            r = resp.tile([GT, OUT_DIM], F32)
            nc.vector.tensor_scalar(
                r,
                poolacc[:, t * OUT_DIM : (t + 1) * OUT_DIM],
                invc_sb[:, t : t + 1],
                None,
                mybir.AluOpType.mult,
            )
            nc.sync.dma_start(out=out[t * GT : (t + 1) * GT, :], in_=r)

    return nc


def kernel(pos, z, batch, emb, W1, b1, W2, b2, W3, b3):
    global LAST_RESULTS
    pos = np.ascontiguousarray(np.asarray(pos), dtype=np.float32)
    z = np.asarray(z)
    batch = np.asarray(batch)
    emb = np.asarray(emb, dtype=np.float32)
    W1 = np.asarray(W1, dtype=np.float32)
    b1 = np.asarray(b1, dtype=np.float32)
    W2 = np.asarray(W2, dtype=np.float32)
    b2 = np.asarray(b2, dtype=np.float32)
    W3 = np.asarray(W3, dtype=np.float32)
    b3 = np.asarray(b3, dtype=np.float32)

    n = pos.shape[0]
    n_graphs = 4096
    batch64 = batch.astype(np.int64)

    # ---- host-side shard planning (graph-aligned node split) ------------
    starts = np.searchsorted(batch64, np.arange(n_graphs + 1))  # [G+1], starts[G]=n
    cnt = np.diff(starts)
    gsplit = [0]
    for c in range(1, N_CORES):
        target = (c * n) // N_CORES
        g = int(np.searchsorted(starts, target))
        if g > 0 and abs(int(starts[g - 1]) - target) <= abs(int(starts[g]) - target):
            g -= 1
        g = max(gsplit[-1], min(g, n_graphs))
        gsplit.append(g)
    gsplit.append(n_graphs)

    node_lo = [int(starts[gsplit[c]]) for c in range(N_CORES)]
    node_hi = [int(starts[gsplit[c + 1]]) for c in range(N_CORES)]
    nc_nodes = [node_hi[c] - node_lo[c] for c in range(N_CORES)]
    n_tiles = (max(nc_nodes) + NT - 1) // NT
    nc_pad = n_tiles * NT
    gc_graphs = [gsplit[c + 1] - gsplit[c] for c in range(N_CORES)]
    ntg = (max(gc_graphs) + GT - 1) // GT

    # ---- per-core inputs -------------------------------------------------
    embW1 = emb @ W1[3:, :]                      # [100, 128]
    w1e = np.concatenate([embW1, W1[:3, :]], axis=0).astype(np.float32)
    w3h = W3.astype(NPBF)
    w3l = (W3 - w3h.astype(np.float32)).astype(NPBF)
    w3hl = np.concatenate([w3h, w3l], axis=1)    # [128, 128] bf16
    iota_col = np.arange(SUB, dtype=np.float32).reshape(SUB, 1)
    f2 = np.broadcast_to(
        np.arange(ntg * GT, dtype=np.float32), (SUB, ntg * GT)
    ).copy()

    inv_full = (1.0 / np.maximum(cnt, 1)).astype(np.float32)

    in_maps = []
    blocal = []  # per-core local graph id per padded node slot (for schedule)
    for c in range(N_CORES):
        lo, hi, m = node_lo[c], node_hi[c], nc_nodes[c]
        posr_c = np.zeros((3, nc_pad), np.float32)
        posr_c[:, :m] = pos[lo:hi].T
        zbf_c = np.zeros((1, nc_pad), NPBF)
        zbf_c[0, :m] = z[lo:hi].astype(NPBF)
        bl = np.full(nc_pad, 1.0e6, np.float32)
        bl[:m] = (batch64[lo:hi] - gsplit[c]).astype(np.float32)
        blocal.append(bl)
        batch2d_c = np.ascontiguousarray(bl.reshape(nc_pad // SUB, SUB).T)
        invc_c = np.ones((GT, ntg), np.float32)
        gcnt = inv_full[gsplit[c] : gsplit[c + 1]]
        pad = np.ones(ntg * GT, np.float32)
        pad[: gcnt.shape[0]] = gcnt
        invc_c[:, :] = pad.reshape(ntg, GT).T
        in_maps.append(
            {
                "posr": posr_c,
                "zbf": zbf_c,
                "batch2d": batch2d_c,
                "w1e": w1e,
                "w2": W2,
                "w3hl": w3hl,
                "b1": b1.reshape(HID, 1),
                "b2": b2.reshape(HID, 1),
                "iota_col": iota_col,
                "f2": f2,
                "invc": invc_c,
            }
        )

    # ---- pooling schedule: union across cores of graph tiles touched ----
    n_sub_total = nc_pad // SUB
    touch = [set() for _ in range(n_sub_total)]  # sub-tile -> graph tiles
    for c in range(N_CORES):
        bl = blocal[c]
        m = nc_nodes[c]
        for s in range(n_sub_total):
            a, b = s * SUB, min((s + 1) * SUB, m)
            if a >= m:
                break
            t_lo = int(bl[a]) // GT
            t_hi = int(bl[b - 1]) // GT
            for t in range(t_lo, t_hi + 1):
                touch[s].add(t)
    sched = []
    for i in range(n_tiles):
        groups = {}
        for s in range(NT // SUB):
            for t in sorted(touch[i * (NT // SUB) + s]):
                groups.setdefault(t, []).append(s)
        sched.append(sorted(groups.items()))

    # ---- build, run, gather ---------------------------------------------
    nc = _build_bass(n_tiles, ntg, sched)
    _split_multi_waits(nc)
    res = run_bass_kernel_spmd(nc, in_maps, list(range(N_CORES)))
    LAST_RESULTS = res

    out = np.zeros((n_graphs, OUT_DIM), np.float32)
    for c in range(N_CORES):
        g0, g1 = gsplit[c], gsplit[c + 1]
        out[g0:g1] = res.results[c]["out"][: g1 - g0]
    out += b3.reshape(1, OUT_DIM)
    return out


# revision 13
# speedup vs baseline: 3.1418x; 3.1418x over previous
"""Trainium2 Bass kernel for an equivariant-GNN style network.

Reference computation (per node, then per-graph mean pool):
    x  = concat([pos, emb[z]])           # [N, 128]
    x1 = relu(x @ W1 + b1)               # [N, 128]
    x2 = relu(x1 @ W2 + b2)              # [N, 128]
    y  = x2 @ W3 + b3                    # [N, 64]
    out[g] = mean over nodes of graph g of y   # [4096, 64]

Strategy (8 NeuronCores, SPMD, one program):
  - Nodes sharded across cores at graph boundaries (batch is sorted): no
    cross-core collective; each core emits partial per-graph sums for its
    own graph range; the host does the final O(G) scatter-add / scale.
  - Feature-major layout ([feat_partitions, nodes_free]), 1024-node
    iterations (two 512 matmuls per layer, one big elementwise op):
      * emb[z] @ W1[3:] folded into one matmul with an on-chip one-hot(z)
        rhs (bf16 PE broadcast of z, exact for z<256, + DVE is_equal);
        pos rows ride in the same rhs -> layer 1 is ONE fp32r matmul.
      * layers 1-3 in fp32r (~2^-11 product rounding, 1 col/cycle at
        N=512).
      * segment pooling: host precomputes an exact bf16 "staircase"
        one-hot [128 nodes, 8 graph slots] per 128-node sub-tile (batch
        is sorted, so a sub-tile spans few graphs); one tiny matmul per
        sub-tile accumulates into a PSUM slot; slots are drained to DRAM
        and the host adds the partials into the right graph rows.
  - 1/count scaling and b3 are applied on the host (O(4096) work).

The walrus in this toolchain only allows ONE sync wait per instruction;
_split_multi_waits() rewrites Tile's multi-wait instructions into NoOp
chains before compilation.
"""

import os
import sys
from contextlib import ExitStack

import numpy as np

if "/opt/trn_rl_repo" not in sys.path:
    sys.path.insert(0, "/opt/trn_rl_repo")

import ml_dtypes
import concourse.bass as bass
import concourse.tile as tile
from concourse import mybir
from concourse.bass_utils import run_bass_kernel_spmd

N_CORES = 8
NTB = 1024      # nodes per iteration (2 PSUM banks per layer)
MM = 512        # matmul moving-dim chunk (fp32 PSUM bank)
SUB = 128       # nodes per sub-tile (pooling granularity, matmul K limit)
GSLOT = 8       # graph slots per sub-tile staircase
CHUNK = 12      # sub-tiles per PSUM pool tile: 3 row-slots x 4 col-slots
N_TYPES = 100
HID = 128
OUT_DIM = 64
F32 = mybir.dt.float32
F32R = mybir.dt.float32r
BF16 = mybir.dt.bfloat16
NPBF = ml_dtypes.bfloat16

LAST_RESULTS = None  # BassKernelResults of the most recent run (for test.py)


def _split_multi_waits(nc):
    """The TPB ISA has one sync-wait slot per instruction and this walrus
    refuses to lower instructions with more. Move all-but-one wait onto
    same-engine NoOps placed immediately before (engines execute their queue
    in order, so the sync semantics are preserved)."""
    n_split = 0
    for f in nc.m.functions:
        for blk in f.blocks:
            insts = list(blk.instructions)
            out = []
            for inst in insts:
                si = inst.sync_info
                if si is not None and si.on_wait and len(si.on_wait) > 1:
                    waits = list(si.on_wait)
                    for j, w in enumerate(waits[:-1]):
                        nop = mybir.InstNoOp(
                            name=f"{inst.name}-w{j}",
                            engine=inst.engine,
                            ins=[],
                            outs=[],
                            sync_info=mybir.SyncInfo(on_wait=[w], on_update=[]),
                        )
                        out.append(nop)
                    inst.sync_info = mybir.SyncInfo(
                        on_wait=[waits[-1]], on_update=list(si.on_update or [])
                    )
                    n_split += 1
                out.append(inst)
            blk.instructions = out
    return n_split


def _build_bass(n_iters: int) -> bass.Bass:
    """Build the single SPMD program (identical for all cores)."""
    nc = bass.Bass()
    nc_pad = n_iters * NTB
    n_sub = NTB // SUB                     # sub-tiles per iteration (8)
    n_chunks = (n_iters * n_sub + CHUNK - 1) // CHUNK

    posr = nc.dram_tensor("posr", [3, nc_pad], F32R, kind="ExternalInput")
    zbf = nc.dram_tensor("zbf", [1, nc_pad], BF16, kind="ExternalInput")
    sg8 = nc.dram_tensor(
        "sg8", [SUB, n_iters, n_sub * GSLOT], BF16, kind="ExternalInput"
    )
    w1e = nc.dram_tensor("w1e", [N_TYPES + 3, HID], F32R, kind="ExternalInput")
    w2 = nc.dram_tensor("w2", [HID, HID], F32R, kind="ExternalInput")
    w3 = nc.dram_tensor("w3", [HID, OUT_DIM], F32R, kind="ExternalInput")
    b1 = nc.dram_tensor("b1", [HID, 1], F32, kind="ExternalInput")
    b2 = nc.dram_tensor("b2", [HID, 1], F32, kind="ExternalInput")
    iota_col = nc.dram_tensor("iota_col", [SUB, 1], F32, kind="ExternalInput")
    out_part = nc.dram_tensor(
        "out_part", [n_chunks * SUB, 4 * OUT_DIM], F32, kind="ExternalOutput"
    )

    with tile.TileContext(nc) as tc, ExitStack() as ctx:
        singles = ctx.enter_context(tc.tile_pool(name="singles", bufs=1))
        inp = ctx.enter_context(tc.tile_pool(name="inp", bufs=3))
        ohp = ctx.enter_context(tc.tile_pool(name="ohp", bufs=2))
        acts = ctx.enter_context(tc.tile_pool(name="acts", bufs=2))
        resp = ctx.enter_context(tc.tile_pool(name="resp", bufs=2))
        ps_zb = ctx.enter_context(tc.tile_pool(name="ps_zb", bufs=1, space="PSUM"))
        ps_x1 = ctx.enter_context(tc.tile_pool(name="ps_x1", bufs=1, space="PSUM"))
        ps_x2 = ctx.enter_context(tc.tile_pool(name="ps_x2", bufs=1, space="PSUM"))
        ps_y = ctx.enter_context(tc.tile_pool(name="ps_y", bufs=1, space="PSUM"))
        ps_pl = ctx.enter_context(tc.tile_pool(name="ps_pl", bufs=1, space="PSUM"))

        # ---- persistent tiles -------------------------------------------
        w1e_sb = singles.tile([N_TYPES + 3, HID], F32R)
        nc.sync.dma_start(out=w1e_sb, in_=w1e[:, :])
        w2_sb = singles.tile([HID, HID], F32R)
        nc.sync.dma_start(out=w2_sb, in_=w2[:, :])
        w3_sb = singles.tile([HID, OUT_DIM], F32R)
        nc.sync.dma_start(out=w3_sb, in_=w3[:, :])
        b1_sb = singles.tile([HID, 1], F32)
        nc.sync.dma_start(out=b1_sb, in_=b1[:, :])
        b2_sb = singles.tile([HID, 1], F32)
        nc.sync.dma_start(out=b2_sb, in_=b2[:, :])
        iota_sb = singles.tile([SUB, 1], F32)
        nc.sync.dma_start(out=iota_sb, in_=iota_col[:, :])
        ones_sb = singles.tile([1, N_TYPES], BF16)
        nc.vector.memset(ones_sb, 1.0)

        pp = None
        for i in range(n_iters):
            # ---- inputs ----------------------------------------------
            oh = ohp.tile([N_TYPES + 3, NTB], F32R)
            nc.sync.dma_start(
                out=oh[N_TYPES : N_TYPES + 3, :], in_=posr[:, i * NTB : (i + 1) * NTB]
            )
            zf_t = inp.tile([1, NTB], BF16, tag="zf")
            nc.sync.dma_start(out=zf_t, in_=zbf[:, i * NTB : (i + 1) * NTB])
            sg_t = inp.tile([SUB, n_sub, GSLOT], BF16, tag="sg")
            nc.sync.dma_start(out=sg_t, in_=sg8[:, i, :])

            # ---- one-hot(z), type-major ------------------------------
            zb = ps_zb.tile([N_TYPES, NTB], F32)
            for h in range(NTB // MM):
                nc.tensor.matmul(
                    zb[:, h * MM : (h + 1) * MM],
                    ones_sb,
                    zf_t[:, h * MM : (h + 1) * MM],
                    start=True,
                    stop=True,
                )
            nc.vector.tensor_scalar(
                oh[:N_TYPES, :], zb, iota_sb[:N_TYPES, :], None, mybir.AluOpType.is_equal
            )

            # ---- layer 1 (embedding fold + pos) ----------------------
            x1p = ps_x1.tile([HID, NTB], F32)
            for h in range(NTB // MM):
                nc.tensor.matmul(
                    x1p[:, h * MM : (h + 1) * MM],
                    w1e_sb,
                    oh[:, h * MM : (h + 1) * MM],
                    start=True,
                    stop=True,
                )
            x1 = acts.tile([HID, NTB], F32R, tag="x1")
            nc.scalar.activation(x1, x1p, mybir.ActivationFunctionType.Relu, bias=b1_sb)

            # ---- layer 2 ---------------------------------------------
            x2p = ps_x2.tile([HID, NTB], F32)
            for h in range(NTB // MM):
                nc.tensor.matmul(
                    x2p[:, h * MM : (h + 1) * MM],
                    w2_sb,
                    x1[:, h * MM : (h + 1) * MM],
                    start=True,
                    stop=True,
                )
            x2 = acts.tile([HID, NTB], F32R, tag="x2")
            nc.scalar.activation(x2, x2p, mybir.ActivationFunctionType.Relu, bias=b2_sb)

            # ---- layer 3, node-major ---------------------------------
            yp = ps_y.tile([SUB, n_sub, OUT_DIM], F32)
            for s in range(n_sub):
                nc.tensor.matmul(
                    yp[:, s, :],
                    x2[:, s * SUB : (s + 1) * SUB],
                    w3_sb,
                    start=True,
                    stop=True,
                )
            y = acts.tile([SUB, n_sub, OUT_DIM], BF16, tag="y")
            nc.vector.tensor_copy(y, yp)

            # ---- pooling: one staircase matmul per sub-tile ----------
            # slot (a, b): matmul M=8 writes partitions [32a, 32a+8) at
            # column block b (PE out base partition must be 0/32/64/96).
            for s in range(n_sub):
                gs = i * n_sub + s          # global sub-tile index
                slot = gs % CHUNK
                if slot == 0:
                    pp = ps_pl.tile([SUB, 4 * OUT_DIM], F32, tag="pp")
                a, bcol = slot % 3, slot // 3
                nc.tensor.matmul(
                    pp[32 * a : 32 * a + GSLOT, bcol * OUT_DIM : (bcol + 1) * OUT_DIM],
                    sg_t[:, s, :],
                    y[:, s, :],
                    start=True,
                    stop=True,
                )
                if slot == CHUNK - 1 or gs == n_iters * n_sub - 1:
                    chunk = gs // CHUNK
                    pc = resp.tile([SUB, 4 * OUT_DIM], F32)
                    nc.scalar.copy(pc, pp)
                    nc.sync.dma_start(
                        out=out_part[chunk * SUB : (chunk + 1) * SUB, :], in_=pc
                    )

    return nc


def _plan(pos, z, batch64, n):
    """Shard planning + host-side pooling metadata."""
    n_graphs = 4096
    starts = np.searchsorted(batch64, np.arange(n_graphs + 1))
    cnt = np.diff(starts)
    gsplit = [0]
    for c in range(1, N_CORES):
        target = (c * n) // N_CORES
        g = int(np.searchsorted(starts, target))
        if g > 0 and abs(int(starts[g - 1]) - target) <= abs(int(starts[g]) - target):
            g -= 1
        g = max(gsplit[-1], min(g, n_graphs))
        gsplit.append(g)
    gsplit.append(n_graphs)
    node_lo = [int(starts[gsplit[c]]) for c in range(N_CORES)]
    node_hi = [int(starts[gsplit[c + 1]]) for c in range(N_CORES)]
    return gsplit, node_lo, node_hi, cnt


def kernel(pos, z, batch, emb, W1, b1, W2, b2, W3, b3):
    global LAST_RESULTS
    pos = np.ascontiguousarray(np.asarray(pos), dtype=np.float32)
    z = np.asarray(z)
    batch = np.asarray(batch)
    emb = np.asarray(emb, dtype=np.float32)
    W1 = np.asarray(W1, dtype=np.float32)
    b1 = np.asarray(b1, dtype=np.float32)
    W2 = np.asarray(W2, dtype=np.float32)
    b2 = np.asarray(b2, dtype=np.float32)
    W3 = np.asarray(W3, dtype=np.float32)
    b3 = np.asarray(b3, dtype=np.float32)

    n = pos.shape[0]
    n_graphs = 4096
    batch64 = batch.astype(np.int64)
    gsplit, node_lo, node_hi, cnt = _plan(pos, z, batch64, n)

    nc_nodes = [node_hi[c] - node_lo[c] for c in range(N_CORES)]
    n_iters = (max(nc_nodes) + NTB - 1) // NTB
    nc_pad = n_iters * NTB
    n_sub_total = nc_pad // SUB
    n_chunks = (n_sub_total + CHUNK - 1) // CHUNK

    embW1 = emb @ W1[3:, :]
    w1e = np.concatenate([embW1, W1[:3, :]], axis=0).astype(np.float32)
    iota_col = np.arange(SUB, dtype=np.float32).reshape(SUB, 1)

    in_maps = []
    g0_sub = np.zeros((N_CORES, n_sub_total), np.int64)  # first graph per sub-tile
    for c in range(N_CORES):
        lo, hi, m = node_lo[c], node_hi[c], nc_nodes[c]
        posr_c = np.zeros((3, nc_pad), np.float32)
        posr_c[:, :m] = pos[lo:hi].T
        zbf_c = np.zeros((1, nc_pad), NPBF)
        zbf_c[0, :m] = z[lo:hi].astype(NPBF)
        bl = np.full(nc_pad, -1, np.int64)
        bl[:m] = batch64[lo:hi] - gsplit[c]
        # staircase one-hot per sub-tile: sg8[p, i, s*8+j] = (bl[node] == g0+j)
        blk = bl.reshape(n_sub_total, SUB)          # [sub, 128]
        g0 = np.where(blk[:, 0] >= 0, blk[:, 0], 0)  # first graph (pad-> dummy 0)
        valid = blk >= 0
        rel = blk - g0[:, None]                      # 0..span-1 for real nodes
        if valid.any():
            span = rel[valid].max() + 1
            assert span <= GSLOT, f"sub-tile spans {span} graphs > {GSLOT}"
        sg = np.zeros((n_sub_total, SUB, GSLOT), np.float32)
        idx_s, idx_p = np.nonzero(valid)
        sg[idx_s, idx_p, rel[idx_s, idx_p]] = 1.0
        # device layout [SUB, n_iters, n_sub*GSLOT]
        sg_dev = np.ascontiguousarray(
            sg.reshape(n_iters, NTB // SUB, SUB, GSLOT)
            .transpose(2, 0, 1, 3)
            .reshape(SUB, n_iters, (NTB // SUB) * GSLOT)
        ).astype(NPBF)
        g0_sub[c] = g0
        in_maps.append(
            {
                "posr": posr_c,
                "zbf": zbf_c,
                "sg8": sg_dev,
                "w1e": w1e,
                "w2": W2,
                "w3": W3,
                "b1": b1.reshape(HID, 1),
                "b2": b2.reshape(HID, 1),
                "iota_col": iota_col,
            }
        )

    # ---- build, run ------------------------------------------------------
    nc = _build_bass(n_iters)
    _split_multi_waits(nc)
    res = run_bass_kernel_spmd(nc, in_maps, list(range(N_CORES)))
    LAST_RESULTS = res

    # ---- host-side gather: scatter-add partials, scale, bias ------------
    sums = np.zeros((n_graphs + GSLOT, OUT_DIM), np.float64)
    for c in range(N_CORES):
        part = res.results[c]["out_part"]            # [n_chunks*128, 4*64]
        n_chunks = part.shape[0] // SUB
        p4 = part.reshape(n_chunks, 4, 32, 4, OUT_DIM)  # [chunk, a-quad, row, b, f]
        # sub-tile gs -> chunk=gs//16, slot=gs%16, a=slot%4, b=slot//4,
        # rows 32a..32a+8 -> p4[chunk, a, 0:8, b, :]
        gs_all = np.arange(n_sub_total)
        ch, sl = gs_all // CHUNK, gs_all % CHUNK
        rows = p4[ch, sl % 3, :GSLOT, sl // 3, :]    # [n_sub_total, 8, 64]
        gidx = (g0_sub[c][:, None] + np.arange(GSLOT)[None, :] + gsplit[c]).ravel()
        np.add.at(sums, gidx, rows.reshape(-1, OUT_DIM))
    out = (
        sums[:n_graphs].astype(np.float32)
        / np.maximum(cnt, 1).astype(np.float32)[:, None]
    )
    out += b3.reshape(1, OUT_DIM)
    return out
